# revision 18
# baseline (speedup 1.0000x reference)
"""Trainium2 Bass kernel for nn_Attention (B=4, P=2048, D=768, H=12, hd=64).

Sharding: 8 cores = 4 batches x 2 half-head-groups (6 heads each).

Schedule v2 (same-chunk AV): PE is the wall (~225us of bf16 matmul).
Everything else is arranged to never stall it and to minimize lead+tail:
  - chunk c = (head-pair p, 512-query block qq): 16 units of
    [128 keys x 1024 queries] scores -> exp -> AV(kc) immediately in the
    SAME chunk (AV accumulates per half-head across units), so the kernel
    tail is just the last norm + last projection instead of a whole AV.
  - exp runs on ACT for most units; 2-4 units per chunk go to DVE as a
    centered Schraudolph bit-trick (int16 affine -> bf16 bits), keeping
    ACT comfortably off the critical path (rel-err ~9e-3, budget 2e-2).
  - norm per (head, qq): DVE copy+recip, DRAM-bounce broadcast, multiply
    on Pool (SBUF-only op; GpSimd has no PSUM port).
  - vsb is tight [128, 16, 390] = 6 heads x (ones-col + 64 v-dims); the
    ones column is memset once on Pool; the v bias is folded into b_proj
    on the host (y += wp @ b_v exactly, since softmax rows sum to 1).
  - lead: no big gpsimd memsets before the PE warm-up; phase A (ft3 all
    blocks + ft0 block 0, cc-outer) tracks the per-cc DMA arrival.

Per-core layouts (host-prepared):
  xT   [768, 2048] bf16  rows 0..767 = x[b].T
  wqk  [768, 768]  bf16  [c, feat]; feat-tile order [q01 k01 q23 k23 q45 k45]
  wv   [768, 384]  bf16  [c, 6 heads x 64 v-dims]
  wp   [384, 768]  bf16  [feat (6 heads x 64), out-features]
  bqk  [128, 6]    f32   per-partition bias per qk feature tile
  bp   [128, 6]    f32   (b_proj + w_proj @ b_v_half) / 2 per out-feature tile
Output:
  yT   [768, 2048] f32   partial (pre pair-sum) transposed projection
"""

import math
import sys
from collections import deque

import numpy as np

if "/opt/trn_rl_repo" not in sys.path:
    sys.path.insert(0, "/opt/trn_rl_repo")

B, P, D = 4, 2048, 768
H, HD = 12, 64
N_CORES = 8
H_LOC = 6
SCALE = HD ** -0.5

CC = 6
FT_COL = {0: 0, 3: 1, 1: 2, 4: 3, 2: 4, 5: 5}  # wqk column-tile by ft
KT = 16
TB = 4
VW = H_LOC * HD        # 384
UNIT = 1024
LN2 = math.log(2.0)
# centered Schraudolph: bits = s*SCALE*128/ln2 + (127*128 + .5 - center)
SCH_A = 128.0 / (LN2 * HD ** 0.5)
SCH_B = 127.0 * 128.0 + 0.5 - 7.36

# units handed to DVE (Schraudolph) per chunk
OFFLOAD_ON = False
OFFLOAD = {c: ((9, 11, 13, 15) if c >= 4 else ((13, 15) if c >= 1 else ()))
           if OFFLOAD_ON else ()
           for c in range(12)}

_PROG = None


def _build_program():
    import concourse.mybir as mybir
    import concourse.tile as tile
    from concourse import bacc

    f32 = mybir.dt.float32
    bf16 = mybir.dt.bfloat16
    i16 = mybir.dt.int16
    AF = mybir.ActivationFunctionType
    ALU = mybir.AluOpType

    nc = bacc.Bacc("TRN2")

    xT = nc.declare_dram_parameter("xT", [768, 2048], bf16, isOutput=False)
    wqk = nc.declare_dram_parameter("wqk", [768, 768], bf16, isOutput=False)
    wv = nc.declare_dram_parameter("wv", [768, VW], bf16, isOutput=False)
    wp = nc.declare_dram_parameter("wp", [384, 768], bf16, isOutput=False)
    bqk = nc.declare_dram_parameter("bqk", [128, 6], f32, isOutput=False)
    bp = nc.declare_dram_parameter("bp", [128, 6], f32, isOutput=False)
    yT = nc.declare_dram_parameter("yT", [768, 2048], f32, isOutput=True)
    import os
    dbg_on = bool(os.environ.get("KDBG"))
    if dbg_on:
        dq = nc.declare_dram_parameter("dbg_qkt", [128, 6 * 2048], bf16,
                                       isOutput=True)
        dv = nc.declare_dram_parameter("dbg_vsb", [128, 16 * 768], bf16,
                                       isOutput=True)
        do = nc.declare_dram_parameter("dbg_otsb", [128, 3 * 2048], bf16,
                                       isOutput=True)
        dosb = nc.declare_dram_parameter("dbg_osb", [128, 512], f32,
                                         isOutput=True)
        drb = nc.declare_dram_parameter("dbg_rb", [64, 512], f32,
                                        isOutput=True)
        drec = nc.declare_dram_parameter("dbg_rec", [1, 512], f32,
                                         isOutput=True)

    with tile.TileContext(nc) as tc:
        with (
            tc.tile_pool(name="persist", bufs=1) as persist,
            tc.tile_pool(name="slabs", bufs=2) as slabs,
            tc.tile_pool(name="norm", bufs=3) as norm,
            tc.tile_pool(name="drs", bufs=4, space="DRAM") as drs,
        ):
            qkt = persist.tile([128, 6, 2048], bf16, tag="qkt")
            vsb = persist.tile([128, KT, H_LOC * 128], bf16, tag="vsb")
            otsb = persist.tile([128, 3, 2048], bf16, tag="otsb")
            bqk_sb = persist.tile([128, 6], f32, tag="bqk_sb")
            bp_sb = persist.tile([128, 6], f32, tag="bp_sb")
            wp_sb = persist.tile([128, 3, 768], bf16, tag="wp_sb")
            xts = [
                persist.tile([128, 2048], bf16, tag=f"xt{i}", name=f"xt{i}")
                for i in range(CC)
            ]
            wqk_sbs = [
                persist.tile([128, 768], bf16, tag=f"wqk{i}", name=f"wqk{i}")
                for i in range(CC)
            ]
            wv_sbs = [
                persist.tile([128, VW], bf16, tag=f"wv{i}", name=f"wv{i}")
                for i in range(CC)
            ]

            # ---- priority-ordered input DMA on both hwdge rings.
            # sync: xt0/2/4 + critical wqk cols, then wqk rest, wp.
            # scalar: xt1/3/5 + critical wqk cols, biases, then wv (needed
            # by the v fills from ~8us).
            for ccx in range(CC):
                eng = nc.sync if ccx % 2 == 0 else nc.scalar
                eng.dma_start(out=xts[ccx],
                              in_=xT[ccx * 128:(ccx + 1) * 128, :])
                eng.dma_start(
                    out=wqk_sbs[ccx][:, 0:256],
                    in_=wqk[ccx * 128:(ccx + 1) * 128, 0:256])
            nc.scalar.dma_start(out=bqk_sb, in_=bqk[:, :])
            nc.scalar.dma_start(out=bp_sb, in_=bp[:, :])
            for ccx in range(CC):
                nc.scalar.dma_start(out=wv_sbs[ccx],
                                    in_=wv[ccx * 128:(ccx + 1) * 128, :])
            for ccx in range(CC):
                nc.sync.dma_start(
                    out=wqk_sbs[ccx][:, 256:768],
                    in_=wqk[ccx * 128:(ccx + 1) * 128, 256:768])
            for fc in range(3):
                nc.sync.dma_start(out=wp_sb[:, fc, :],
                                  in_=wp[fc * 128:(fc + 1) * 128, :])

            # vsb per head: [ones col | 63 zeros | 64 v-dims] so the AV
            # output has denom at partition 0 and feats at 64..127 (the
            # HW-proven layout: recip reads partition 0, mul reads 64:128)
            nc.gpsimd.memset(
                vsb.rearrange("p a (h g) -> p a h g", g=128)[:, :, :, 0:1],
                1.0)
            nc.gpsimd.memset(
                vsb.rearrange("p a (h g) -> p a h g", g=128)[:, :, :, 1:64],
                0.0)

            # pre-warm the exp ACT table during the DMA lead
            warmup = norm.tile([1, 1], f32, tag="warmup", bufs=1)
            nc.vector.memset(warmup, 0.0)
            nc.scalar.activation(out=warmup, in_=warmup, func=AF.Exp)
            # PE p-state warm-up: throwaway matmuls on a DVE-zeroed tile
            wtile = persist.tile([128, 640], bf16, tag="wtile")
            nc.vector.memset(wtile, 0.0)

            # ===== phase A: ft0 (q pair0) block 0 + ft3 (k pair0) all,
            # cc-outer over PSUM groups tracking per-cc DMA arrival
            psA_ctx = tc.tile_pool(name="psA", bufs=8, space="PSUM")
            psA = psA_ctx.__enter__()
            qpA = {}
            for ft, tb in ((3, 0), (3, 1), (3, 2), (3, 3), (0, 0)):
                qpA[(ft, tb)] = psA.tile([128, 512], f32, tag="qpA",
                                         name=f"qpA{ft}_{tb}")
            for _ in range(5):
                nc.tensor.matmul(
                    qpA[(3, 0)],
                    wtile[:, 0:128],
                    wtile[:, 128:640],
                    start=True,
                    stop=True,
                )
            for ccx in range(CC):
                for ft, tb in ((3, 0), (3, 1), (3, 2), (3, 3), (0, 0)):
                    fc_ = FT_COL[ft]
                    nc.tensor.matmul(
                        qpA[(ft, tb)],
                        wqk_sbs[ccx][:, fc_ * 128:(fc_ + 1) * 128],
                        xts[ccx][:, tb * 512:(tb + 1) * 512],
                        start=(ccx == 0),
                        stop=(ccx == CC - 1),
                    )
            for ft, tb in ((3, 0), (0, 0), (3, 1), (3, 2), (3, 3)):
                nc.vector.tensor_scalar_add(
                    out=qkt[:, ft, tb * 512:(tb + 1) * 512],
                    in0=qpA[(ft, tb)],
                    scalar1=bqk_sb[:, ft:ft + 1],
                )
            psA_ctx.__exit__(None, None, None)

            # ===== phase B =====
            with (
                tc.tile_pool(name="psum_s", bufs=2, space="PSUM") as psum_s,
                tc.tile_pool(name="psum_o", bufs=2, space="PSUM") as psum_o,
                tc.tile_pool(name="psum_f", bufs=2, space="PSUM") as psum_f,
            ):
                # ---------- fill jobs ----------
                fills = deque()

                def v_job(pt):
                    """v projection for one token tile: 2 steps, 6 matmuls"""
                    holder = {}

                    def s1():
                        vp = psum_f.tile([128, 512], f32, tag="fp",
                                         name=f"vp{pt}")
                        holder["vp"] = vp
                        for ccx in range(3):
                            nc.tensor.matmul(
                                vp[:, 0:VW],
                                xts[ccx][:, pt * 128:(pt + 1) * 128],
                                wv_sbs[ccx],
                                start=(ccx == 0),
                                stop=False,
                            )

                    def s2():
                        vp = holder["vp"]
                        for ccx in range(3, CC):
                            nc.tensor.matmul(
                                vp[:, 0:VW],
                                xts[ccx][:, pt * 128:(pt + 1) * 128],
                                wv_sbs[ccx],
                                start=False,
                                stop=(ccx == CC - 1),
                            )
                        vdst = vsb.rearrange(
                            "p a (h g) -> p a h g", g=128)[:, pt]
                        nc.vector.tensor_copy(
                            out=vdst[:, :, 64:128],
                            in_=vp[:, 0:VW].rearrange(
                                "p (h c) -> p h c", c=HD),
                        )

                    return [s1, s2]

                def qk_job(ft, tb):
                    """leftover qk projection feature tile: 2 steps"""
                    holder = {}

                    def s1():
                        qp = psum_f.tile([128, 512], f32, tag="fp",
                                         name=f"qp{ft}_{tb}")
                        holder["qp"] = qp
                        fc_ = FT_COL[ft]
                        for ccx in range(3):
                            nc.tensor.matmul(
                                qp,
                                wqk_sbs[ccx][:, fc_ * 128:(fc_ + 1) * 128],
                                xts[ccx][:, tb * 512:(tb + 1) * 512],
                                start=(ccx == 0),
                                stop=False,
                            )

                    def s2():
                        qp = holder["qp"]
                        fc_ = FT_COL[ft]
                        for ccx in range(3, CC):
                            nc.tensor.matmul(
                                qp,
                                wqk_sbs[ccx][:, fc_ * 128:(fc_ + 1) * 128],
                                xts[ccx][:, tb * 512:(tb + 1) * 512],
                                start=False,
                                stop=(ccx == CC - 1),
                            )
                        nc.vector.tensor_scalar_add(
                            out=qkt[:, ft, tb * 512:(tb + 1) * 512],
                            in0=qp,
                            scalar1=bqk_sb[:, ft:ft + 1],
                        )

                    return [s1, s2]

                def proj_job(of, tb):
                    """output projection tile: 1 step (3 MMs + evac + DMA)"""
                    def s1():
                        pp = psum_f.tile([128, 512], f32, tag="fp",
                                         name=f"pp{of}_{tb}")
                        for fc in range(3):
                            nc.tensor.matmul(
                                pp,
                                wp_sb[:, fc, of * 128:(of + 1) * 128],
                                otsb[:, fc, tb * 512:(tb + 1) * 512],
                                start=(fc == 0),
                                stop=(fc == 2),
                            )
                        ysl = norm.tile([128, 512], f32, tag="ysl")
                        nc.vector.tensor_scalar_add(
                            out=ysl, in0=pp, scalar1=bp_sb[:, of:of + 1])
                        nc.sync.dma_start(
                            out=yT[of * 128:(of + 1) * 128,
                                   tb * 512:(tb + 1) * 512],
                            in_=ysl,
                        )

                    return [s1]

                # fill order: v first (needed by same-chunk AV from chunk 0),
                # then leftover qk by deadline; proj jobs appended when their
                # token block is fully normalized.
                for pt in range(2, KT):
                    fills.extend(v_job(pt))
                for tb in range(1, TB):
                    fills.extend(qk_job(0, tb))
                for tb in range(TB):
                    fills.extend(qk_job(4, tb))
                for tb in range(TB):
                    fills.extend(qk_job(1, tb))
                for tb in range(TB):
                    fills.extend(qk_job(5, tb))
                for tb in range(TB):
                    fills.extend(qk_job(2, tb))

                norm_count = {}

                def ot_norm(ph, qq, op):
                    """copy+recip on DVE, DRAM-bounce broadcast, mul on Pool"""
                    osb = norm.tile([128, 512], f32, tag="osb")
                    nc.vector.tensor_copy(out=osb, in_=op)
                    rec = norm.tile([1, 512], f32, tag="rec")
                    rsc = norm.tile([1, 512], f32, tag="rsc")
                    nc.vector.reciprocal_approx_accurate(
                        out=rec, in_=osb[0:1, :], scratch=rsc)
                    dsc = drs.tile([1, 512], f32, tag="dsc")
                    nc.sync.dma_start(out=dsc, in_=rec)
                    rb = norm.tile([128, 512], f32, tag="rb")
                    nc.gpsimd.dma_start(out=rb[64:128, :],
                                        in_=dsc.partition_broadcast(64))
                    pb = 64 * (ph % 2)
                    nc.vector.tensor_mul(
                        out=otsb[pb:pb + 64, ph // 2,
                                 qq * 512:(qq + 1) * 512],
                        in0=osb[64:128, :],
                        in1=rb[64:128, :],
                    )
                    if dbg_on and ph == 0 and qq == 0:
                        nc.scalar.dma_start(out=dosb[:, :], in_=osb)
                        nc.scalar.dma_start(out=drb[:, :], in_=rb[64:128, :])
                        nc.scalar.dma_start(out=drec[:, :], in_=rec)
                    norm_count[qq] = norm_count.get(qq, 0) + 1
                    if norm_count[qq] == H_LOC:
                        for of in range(6):
                            fills.extend(proj_job(of, qq))

                def score_mm(p, qq, sp, kt, hd, off):
                    pb = 64 * hd
                    qlo = qq * 512
                    nc.tensor.matmul(
                        sp[:, off:off + 512],
                        qkt[pb:pb + 64, 3 + p, kt * 128:(kt + 1) * 128],
                        qkt[pb:pb + 64, p, qlo:qlo + 512],
                        start=True,
                        stop=True,
                    )

                def av_mm(p, slab, op_, kc):
                    for hd in range(2):
                        ph = 2 * p + hd
                        nc.tensor.matmul(
                            op_[hd],
                            vsb[:, kc, ph * 128:(ph + 1) * 128],
                            slab.rearrange("p a b -> p (a b)")[
                                :, (2 * kc + hd) * 512:(2 * kc + hd + 1) * 512],
                            start=(kc == 0),
                            stop=(kc == KT - 1),
                        )

                def drain(n):
                    for _ in range(n):
                        if fills:
                            fills.popleft()()

                def emit_chunk(c):
                    p, qq = c // 4, c % 4
                    slab = slabs.tile([128, 2 * KT, 512], bf16, tag="slab")
                    slab_flat = slab.rearrange("p a b -> p (a b)")
                    op_ = {
                        hd: psum_o.tile([128, 512], f32, tag="op",
                                        name=f"op{c}_{hd}")
                        for hd in range(2)
                    }
                    off_dve = OFFLOAD[c]
                    for u in range(KT):
                        sp = psum_s.tile([128, UNIT], f32, tag="sp")
                        score_mm(p, qq, sp, u, 0, 0)
                        score_mm(p, qq, sp, u, 1, 512)
                        dst = slab_flat[:, u * UNIT:(u + 1) * UNIT]
                        if u in off_dve:
                            nc.vector.tensor_scalar(
                                out=dst.bitcast(i16),
                                in0=sp,
                                scalar1=SCH_A,
                                scalar2=SCH_B,
                                op0=ALU.mult,
                                op1=ALU.add,
                            )
                        else:
                            nc.scalar.activation(
                                out=dst, in_=sp, func=AF.Exp, scale=SCALE)
                        if u > 0:
                            av_mm(p, slab, op_, u - 1)
                        if c == 0:
                            drain(2)
                        elif u % 2 == 0:
                            drain(1)
                    # last AV + norms (psum_o freed early via the osb copy)
                    av_mm(p, slab, op_, KT - 1)
                    for hd in range(2):
                        ot_norm(2 * p + hd, qq, op_[hd])

                # v fills 0/1 head-start before chunk 0
                for step in v_job(0) + v_job(1):
                    step()
                for c in range(12):
                    emit_chunk(c)
                while fills:
                    drain(1)
                if dbg_on:
                    nc.scalar.dma_start(
                        out=dq[:, :], in_=qkt.rearrange("p a b -> p (a b)"))
                    nc.scalar.dma_start(
                        out=dv[:, :], in_=vsb.rearrange("p a b -> p (a b)"))
                    nc.scalar.dma_start(
                        out=do[:, :], in_=otsb.rearrange("p a b -> p (a b)"))

    nc.finalize()
    return nc


def _get_program():
    global _PROG
    if _PROG is None:
        _PROG = _build_program()
    return _PROG


def _prep_core_inputs(x, w_qkv, b_qkv, w_proj, b_proj, core):
    b, half = core // 2, core % 2
    heads = np.arange(H_LOC) + H_LOC * half
    d = np.arange(HD)

    import ml_dtypes
    bft = ml_dtypes.bfloat16
    xT = np.ascontiguousarray(x[b].T.astype(bft))

    # torch reshape quirk: feature (t, d, h) -> row t*768 + d*12 + h
    qk_rows = np.empty(768, np.int64)
    for j in range(3):
        for hp in range(2):
            hh = heads[2 * j + hp]
            base = j * 128 + hp * 64
            qk_rows[base:base + 64] = d * 12 + hh
            qk_rows[384 + base:384 + base + 64] = 768 + d * 12 + hh
    bqk = np.ascontiguousarray(b_qkv[qk_rows].reshape(6, 128).T)
    # wqk DRAM column-tile order [ft0 ft3 ft1 ft4 ft2 ft5] so the
    # phase-A-critical ft0/ft3 columns are one contiguous leading slice
    col_order = np.concatenate([np.arange(128) + 128 * ft
                                for ft in (0, 3, 1, 4, 2, 5)])
    wqk = np.ascontiguousarray(w_qkv[qk_rows[col_order]].T.astype(bft))

    wv = np.empty((768, VW), bft)
    bv = np.empty(VW, np.float64)
    vcols = np.empty(VW, np.int64)
    for i in range(H_LOC):
        rows = 1536 + d * 12 + heads[i]
        wv[:, HD * i:HD * i + HD] = w_qkv[rows].T.astype(bft)
        bv[HD * i:HD * i + HD] = b_qkv[rows]
        vcols[HD * i:HD * i + HD] = 64 * heads[i] + d

    wp = np.empty((384, 768), bft)
    for i in range(H_LOC):
        cols = 64 * heads[i] + d
        wp[64 * i:64 * i + 64] = w_proj[:, cols].T
    # fold this half's v-bias through the projection: softmax rows sum to
    # one, so y += w_proj[:, half cols] @ b_v exactly
    bp_eff = b_proj * 0.5 + w_proj[:, vcols].astype(np.float64) @ bv
    bp = np.ascontiguousarray(bp_eff.astype(np.float32).reshape(6, 128).T)

    return {
        "xT": xT,
        "wqk": wqk,
        "wv": np.ascontiguousarray(wv),
        "wp": np.ascontiguousarray(wp),
        "bqk": bqk,
        "bp": bp,
    }


def _run(inputs, trace=False, **kw):
    from concourse.bass_utils import run_bass_kernel_spmd

    nc = _get_program()
    x = np.asarray(inputs["x"], np.float32)
    w_qkv = np.asarray(inputs["w_qkv"], np.float32)
    b_qkv = np.asarray(inputs["b_qkv"], np.float32)
    w_proj = np.asarray(inputs["w_proj"], np.float32)
    b_proj = np.asarray(inputs["b_proj"], np.float32)

    in_maps = [
        _prep_core_inputs(x, w_qkv, b_qkv, w_proj, b_proj, c)
        for c in range(N_CORES)
    ]
    res = run_bass_kernel_spmd(nc, in_maps, list(range(N_CORES)),
                               trace=trace, **kw)

    out = np.empty((B, P, D), np.float32)
    for b in range(B):
        yt = res.results[2 * b]["yT"] + res.results[2 * b + 1]["yT"]
        out[b] = yt.T
    return out, res


def kernel(**inputs):
    out, _ = _run(inputs)
    return out
